# revision 1
# baseline (speedup 1.0000x reference)
"""GAT 2-layer kernel for 8 Trainium2 NeuronCores (SPMD via bass/Tile).

Strategy:
  - dst-shard nodes across 8 cores; edges grouped by owner core, then by
    128-wide local dst block, then by src-table half, sorted by src.
  - Per layer a DRAM "table" holds one 512B row per node:
      [128 x bf16 feat | el0 el1 er0 er1 f32 | pad] = 128 f32 cols.
    fc is data-parallel over node shards; AllGather replicates the table.
  - Edge phase: per dst block, TWO batched dma_gather ops (one per table
    half, int16 local indices) fetch all edge rows at once; compute
    ex = exp(leakyrelu(el[src]+er[dst])), scale gathered feats, aggregate
    with one-hot matmuls (Pm^T @ [G*ex | ex]) accumulating in PSUM;
    normalize by the z columns afterwards.
  - er[dst] dispersal per edge via one-hot PmT matmuls built cheaply with
    a 4x-mode tensor_scalar; DVE ops are pair-packed for 2x mode.
  - Attention logits el/er are folded into the fc matmul via W@a products.
"""

import sys

import numpy as np

sys.path.insert(0, "/opt/trn_rl_repo")

import ml_dtypes

# problem sizes (overridable via configure() for scaled-down testing)
N = 50000
E = 800000
IN, HID, OUT = 128, 64, 64
H = 2
NEG = 0.2
NCORES = 8
P = 128
NSHARD = N // NCORES
NBLK = (NSHARD + P - 1) // P
RSLICE = NBLK * P + P         # table rows per rank slice (1 pad + nodes + junk)
TROWS = RSLICE * NCORES
HROWS = TROWS // 2            # rows per gather half (must fit int16)
TCOLS = 128                   # table row: 512B
PAD_EL = -100.0


def configure(n, e):
    """Adjust module globals for a scaled-down test configuration."""
    global N, E, NSHARD, NBLK, RSLICE, TROWS, HROWS
    N, E = n, e
    NSHARD = N // NCORES
    NBLK = (NSHARD + P - 1) // P
    RSLICE = NBLK * P + P
    TROWS = RSLICE * NCORES
    HROWS = TROWS // 2


def _row_of(node):
    return RSLICE * (node // NSHARD) + 1 + (node % NSHARD)


def preprocess(a):
    bf16 = ml_dtypes.bfloat16
    f32 = np.float32
    h, src, dst = a["h"], a["src"].astype(np.int64), a["dst"].astype(np.int64)
    assert HROWS <= 32768

    def rhs_pack(W, al, ar):
        k = W.shape[0]
        Wr = W.reshape(k, H, W.shape[1] // H)
        wal = np.einsum("ihj,hj->ih", Wr, al)
        war = np.einsum("ihj,hj->ih", Wr, ar)
        return np.concatenate([W, wal, war], axis=1).astype(bf16)

    rhsW1 = rhs_pack(a["W1"].astype(f32), a["al1"].astype(f32), a["ar1"].astype(f32))
    rhsW2 = rhs_pack(a["W2"].astype(f32), a["al2"].astype(f32), a["ar2"].astype(f32))
    lin1_bp = a["lin1_b"].astype(f32) + a["b1"].astype(f32) @ a["lin1_W"].astype(f32)
    lin2_bp = a["lin2_b"].astype(f32) + a["b2"].astype(f32) @ a["lin2_W"].astype(f32)

    core = dst // NSHARD
    local = dst - core * NSHARD
    blk = local // P
    dloc = local % P
    grow = _row_of(src)
    half = grow // HROWS
    lrow = grow - half * HROWS
    order = np.lexsort((src, half, blk, core))
    core_s, blk_s, half_s = core[order], blk[order], half[order]
    dloc_s, lrow_s = dloc[order], lrow[order]

    counts = np.zeros((NCORES, NBLK, 2), np.int64)
    np.add.at(counts, (core_s, blk_s, half_s), 1)
    sbh = -(-counts.max(axis=0) // P)          # [NBLK, 2] tiles per (blk, half)
    for b in range(NBLK):
        if sbh[b].sum() == 0:
            sbh[b, 0] = 1
    tiles_total = int(sbh.sum())
    slots_total = tiles_total * P
    tile_base = np.zeros((NBLK, 2), np.int64)  # global tile index of (b, h)
    acc = 0
    for b in range(NBLK):
        for hh in range(2):
            tile_base[b, hh] = acc
            acc += sbh[b, hh]

    idx_arr = np.zeros((NCORES, slots_total), np.int16)
    dloc_arr = np.zeros((NCORES, slots_total), np.float32)
    for c in range(NCORES):
        m = core_s == c
        cb, ch = blk_s[m], half_s[m]
        cd, cl = dloc_s[m], lrow_s[m]
        cnt = counts[c]                        # [NBLK, 2]
        off = 0
        for b in range(NBLK):
            for hh in range(2):
                n_ = int(cnt[b, hh])
                if n_:
                    s0 = int(tile_base[b, hh]) * P
                    idx_arr[c, s0:s0 + n_] = cl[off:off + n_]
                    dloc_arr[c, s0:s0 + n_] = cd[off:off + n_]
                    off += n_
    # wrapped int16 index layout: index j -> [j%16 (+16r), j//16]
    w16 = idx_arr.reshape(NCORES, slots_total // 16, 16).transpose(0, 2, 1)
    idx16 = np.tile(w16, (1, 8, 1)).astype(np.int16)
    dloc_pt = np.ascontiguousarray(
        dloc_arr.reshape(NCORES, tiles_total, P).transpose(0, 2, 1)).astype(bf16)
    dloc2_pt = np.ascontiguousarray(np.repeat(dloc_pt, 2, axis=2))

    NCOLS = NBLK * P
    hT = np.zeros((NCORES, IN, NCOLS), bf16)
    for c in range(NCORES):
        hT[c, :, :NSHARD] = h[c * NSHARD:(c + 1) * NSHARD].T.astype(bf16)

    padrow = np.zeros((1, TCOLS), f32)
    padrow[0, 64] = PAD_EL
    padrow[0, 65] = PAD_EL
    iota2d = np.tile(np.arange(P, dtype=f32), (P, 1)).astype(bf16)

    shared = dict(rhsW1=rhsW1, rhsW2=rhsW2,
                  lin1W=a["lin1_W"].astype(bf16), lin2W=a["lin2_W"].astype(bf16),
                  b1col=lin1_bp.reshape(HID, 1).astype(f32),
                  b2row=np.tile(lin2_bp[None, :], (P, 1)).astype(f32),
                  padrow=padrow, iota2d=iota2d,
                  iotaP=np.arange(P, dtype=f32).reshape(P, 1))
    per_core = [dict(hT=np.ascontiguousarray(hT[c]), idx16=idx16[c],
                     dloc2=dloc2_pt[c],
                     dloc_flat=dloc_arr[c].reshape(1, -1).astype(bf16))
                for c in range(NCORES)]
    return shared, per_core, sbh


def build(sbh, repeat=1, variant="all"):
    import concourse.bass as bass
    import concourse.bacc as bacc
    import concourse.mybir as mybir
    from concourse import tile

    dt = mybir.dt
    NBLKS = NBLK
    sbh = np.asarray(sbh)
    tiles_total = int(sbh.sum())
    slots_total = tiles_total * P
    tile_base = np.zeros((NBLK, 2), np.int64)
    acc = 0
    for b in range(NBLK):
        for hh in range(2):
            tile_base[b, hh] = acc
            acc += sbh[b, hh]
    NCOLS = NBLK * P

    nc = bacc.Bacc("TRN2", target_bir_lowering=False, debug=False,
                   num_devices=NCORES)

    def din(name, shape, dty):
        return nc.dram_tensor(name, shape, dty, kind="ExternalInput").ap()

    d_hT = din("hT", [IN, NCOLS], dt.bfloat16)
    d_idx16 = din("idx16", [P, slots_total // 16], dt.int16)
    d_dloc2 = din("dloc2", [P, 2 * tiles_total], dt.bfloat16)
    d_rhsW1 = din("rhsW1", [IN, 132], dt.bfloat16)
    d_rhsW2 = din("rhsW2", [HID, 132], dt.bfloat16)
    d_lin1W = din("lin1W", [2 * HID, HID], dt.bfloat16)
    d_lin2W = din("lin2W", [2 * OUT, OUT], dt.bfloat16)
    d_b1col = din("b1col", [HID, 1], dt.float32)
    d_b2row = din("b2row", [P, OUT], dt.float32)
    d_padrow = din("padrow", [1, TCOLS], dt.float32)
    d_iota = din("iota2d", [P, P], dt.bfloat16)
    d_iotaP = din("iotaP", [P, 1], dt.float32)
    d_dlocflat = din("dloc_flat", [1, slots_total], dt.bfloat16)
    d_out = nc.dram_tensor("out", [NSHARD, OUT], dt.float32,
                           kind="ExternalOutput").ap()

    with tile.TileContext(nc) as tc:
        with (
            tc.tile_pool(name="const", bufs=1) as cpool,
            tc.tile_pool(name="dram", bufs=1, space="DRAM") as dpool,
            tc.tile_pool(name="big", bufs=1) as bigpool,
            tc.tile_pool(name="lhs", bufs=1) as lhspool,
        ):
            idx16_t = cpool.tile([P, slots_total // 16], dt.int16)
            dloc2_t = cpool.tile([P, 2 * tiles_total], dt.bfloat16)
            iota_t = cpool.tile([P, P], dt.bfloat16)
            iotaP_t = cpool.tile([P, 1], dt.float32)
            b2_t = cpool.tile([P, OUT], dt.float32)
            b1_t = cpool.tile([HID, 1], dt.float32)
            pad_t = cpool.tile([1, TCOLS], dt.float32)
            rhs1_t = cpool.tile([IN, 132], dt.bfloat16)
            rhs2_t = cpool.tile([HID, 132], dt.bfloat16)
            l1w_t = cpool.tile([2 * HID, HID], dt.bfloat16)
            l2w_t = cpool.tile([2 * OUT, OUT], dt.bfloat16)
            for t, d in ((idx16_t, d_idx16), (dloc2_t, d_dloc2),
                         (iota_t, d_iota), (iotaP_t, d_iotaP),
                         (b2_t, d_b2row), (b1_t, d_b1col), (pad_t, d_padrow),
                         (rhs1_t, d_rhsW1), (rhs2_t, d_rhsW2),
                         (l1w_t, d_lin1W), (l2w_t, d_lin2W)):
                nc.sync.dma_start(t[:], d[:])

            hT_t = lhspool.tile([IN, NCOLS], dt.bfloat16, tag="lhs")
            nc.sync.dma_start(hT_t[:], d_hT[:])

            er1_sb = bigpool.tile([P, NBLKS, H], dt.bfloat16)
            er2_sb = bigpool.tile([P, NBLKS, H], dt.bfloat16)
            t1 = bigpool.tile([P, NBLKS, P], dt.bfloat16)
            t1T = bigpool.tile([P, NCOLS], dt.bfloat16)
            t2 = bigpool.tile([P, NBLKS, P], dt.bfloat16)
            t2T = t1T

            def fc_phase(lhs_tile, rhs_t, slice_d, er_sb, pool_fc, pool_stg):
                nc.sync.dma_start(slice_d[0:1, :], pad_t[:])
                for nb in range(NBLKS):
                    ps = pool_fc.tile([P, 132], dt.float32, tag="fcps")
                    nc.tensor.matmul(ps[:], lhs_tile[:, nb * P:(nb + 1) * P],
                                     rhs_t[:], start=True, stop=True)
                    stg = pool_stg.tile([P, TCOLS], dt.float32, tag="fcstg")
                    nc.scalar.activation(
                        stg[:].bitcast(dt.bfloat16)[:, 0:128], ps[:, 0:128],
                        mybir.ActivationFunctionType.Copy)
                    nc.vector.tensor_copy(stg[:, 64:68], ps[:, 128:132])
                    nc.vector.tensor_copy(er_sb[:, nb, :], ps[:, 130:132])
                    nc.sync.dma_start(
                        slice_d[1 + nb * P:1 + (nb + 1) * P, :], stg[:])

            def edge_phase(tab_d, t_stg, er_sb, pool_g, pool_e, pool_ps,
                           pool_eps, post_block=None):
                for b in range(NBLKS):
                    S0, S1 = int(sbh[b, 0]), int(sbh[b, 1])
                    S = S0 + S1
                    t0 = int(tile_base[b, 0])
                    G = pool_g.tile([P, S, TCOLS], dt.float32, tag="g")
                    for hh, (sh, toff) in enumerate(((S0, 0), (S1, S0))):
                        if sh == 0:
                            continue
                        s0 = (t0 + toff) * P
                        nc.gpsimd.dma_gather(
                            G[:, toff:toff + sh, :],
                            tab_d[hh * HROWS:(hh + 1) * HROWS, :],
                            idx16_t[:, s0 // 16:(s0 + sh * P) // 16],
                            sh * P, sh * P, TCOLS, single_packet=False)
                    dlocF = pool_e.tile([P, S * P], dt.bfloat16, tag="dlocF")
                    nc.sync.dma_start(
                        dlocF[:],
                        d_dlocflat[0:1, t0 * P:(t0 + S) * P]
                        .partition_broadcast(P).squeeze(1))
                    PmT = pool_e.tile([P, S, P], dt.bfloat16, tag="pmt")
                    nc.vector.tensor_scalar(
                        PmT[:].rearrange("p s e -> p (s e)"), dlocF[:],
                        iotaP_t[:, 0:1], None, mybir.AluOpType.is_equal)
                    er_ps = pool_eps.tile([P, H * S], dt.float32, tag="erps")
                    for s in range(S):
                        nc.tensor.matmul(er_ps[:, H * s:H * (s + 1)],
                                         PmT[:, s, :], er_sb[:, b, :],
                                         start=True, stop=True)
                    ex = pool_e.tile([P, S, H], dt.float32, tag="ex")
                    tmp = pool_e.tile([P, S, H], dt.float32, tag="tmp")
                    nc.vector.tensor_tensor(
                        ex[:], G[:, :, 64:66],
                        er_ps[:].rearrange("p (s h) -> p s h", s=S),
                        mybir.AluOpType.add)
                    nc.vector.tensor_scalar(tmp[:], ex[:], NEG, None,
                                            mybir.AluOpType.mult)
                    nc.vector.tensor_tensor(ex[:], ex[:], tmp[:],
                                            mybir.AluOpType.max)
                    stg = pool_e.tile([P, S, 132], dt.bfloat16, tag="stg")
                    exd = stg[:, :, 128:132].rearrange(
                        "p s (h two) -> p s h two", h=H)
                    for k in range(2):
                        nc.scalar.activation(exd[:, :, :, k], ex[:],
                                             mybir.ActivationFunctionType.Exp)
                    Gb = G[:].bitcast(dt.bfloat16)
                    for hh in range(H):
                        nc.vector.tensor_tensor(
                            stg[:, :, hh * 64:(hh + 1) * 64]
                            .rearrange("p s (c two) -> p s c two", two=2),
                            Gb[:, :, hh * 64:(hh + 1) * 64]
                            .rearrange("p s (c two) -> p s c two", two=2),
                            stg[:, :, 128 + 2 * hh:130 + 2 * hh]
                            .rearrange("p s (c two) -> p s c two", two=2)
                            .broadcast_to([P, S, 32, 2]),
                            mybir.AluOpType.mult)
                    Pm = pool_e.tile([P, S, P], dt.bfloat16, tag="pm")
                    nc.vector.tensor_tensor(
                        Pm[:].rearrange("p s (d two) -> p s d two", two=2),
                        dloc2_t[:, 2 * t0:2 * (t0 + S)]
                        .rearrange("p (s two) -> p s two", two=2)
                        .unsqueeze(2).broadcast_to([P, S, 64, 2]),
                        iota_t[:].rearrange("p (d two) -> p d two", two=2)
                        .unsqueeze(1).broadcast_to([P, S, 64, 2]),
                        mybir.AluOpType.is_equal)
                    acc = pool_ps.tile([P, 132], dt.float32, tag="acc")
                    for s in range(S):
                        nc.tensor.matmul(acc[:], Pm[:, s, :], stg[:, s, :],
                                         start=(s == 0), stop=(s == S - 1))
                    rz = pool_e.tile([P, H], dt.float32, tag="rz")
                    nc.vector.tensor_scalar(
                        rz[:],
                        acc[:, 128:132].rearrange("p (h two) -> p h two", h=H)
                        [:, :, 0:1].squeeze(2),
                        1e-30, None, mybir.AluOpType.add)
                    nc.vector.reciprocal(rz[:], rz[:])
                    for hh in range(H):
                        nc.vector.tensor_scalar(
                            t_stg[:, b, hh * 64:(hh + 1) * 64],
                            acc[:, hh * 64:(hh + 1) * 64],
                            rz[:, hh:hh + 1], None, mybir.AluOpType.mult)
                    if post_block is not None:
                        post_block(b)

            def gather_only(tab_d, pool_g, rep):
                for b in range(NBLKS):
                    S0, S1 = int(sbh[b, 0]), int(sbh[b, 1])
                    S = S0 + S1
                    t0 = int(tile_base[b, 0])
                    G = pool_g.tile([P, S, TCOLS], dt.float32, tag="g")
                    for hh, (sh, toff) in enumerate(((S0, 0), (S1, S0))):
                        if sh == 0:
                            continue
                        s0 = (t0 + toff) * P
                        nc.gpsimd.dma_gather(
                            G[:, toff:toff + sh, :],
                            tab_d[hh * HROWS:(hh + 1) * HROWS, :],
                            idx16_t[:, s0 // 16:(s0 + sh * P) // 16],
                            sh * P, sh * P, TCOLS, single_packet=False)
                    jk = pool_g.tile([P, 1], dt.float32, tag="jk")
                    nc.vector.tensor_copy(jk[:], G[:, 0, 0:1])

            def one_layer(lhs_tile, rhs_t, er_sb, t_stg, tag, do_edge=True,
                          do_ag=True, post_block=None, post_pools=()):
                slice_d = dpool.tile([RSLICE, TCOLS], dt.float32,
                                     tag=f"slice{tag}")
                tab_d = dpool.tile([TROWS, TCOLS], dt.float32,
                                   addr_space="Shared", tag=f"tab{tag}")
                with (
                    tc.tile_pool(name=f"fcps{tag}", bufs=2,
                                 space="PSUM") as fcps,
                    tc.tile_pool(name=f"fcstg{tag}", bufs=3) as fcstg,
                ):
                    fc_phase(lhs_tile, rhs_t, slice_d, er_sb, fcps, fcstg)
                if not do_ag:
                    return None
                nc.gpsimd.collective_compute(
                    "AllGather", mybir.AluOpType.bypass,
                    replica_groups=[list(range(NCORES))],
                    ins=[slice_d.opt()], outs=[tab_d.opt()])
                if not do_edge:
                    return None
                with (
                    tc.tile_pool(name=f"gpool{tag}", bufs=4) as gpool,
                    tc.tile_pool(name=f"epool{tag}", bufs=5) as epool,
                    tc.tile_pool(name=f"pspool{tag}", bufs=3,
                                 space="PSUM") as psp,
                    tc.tile_pool(name=f"epspool{tag}", bufs=3,
                                 space="PSUM") as epsp,
                ):
                    if do_edge == "gath":
                        gather_only(tab_d, gpool, tag)
                    else:
                        edge_phase(tab_d, t_stg, er_sb, gpool, epool, psp,
                                   epsp, post_block=post_block)
                return tab_d

            if variant in ("edge1", "gath1", "ag", "fc"):
                for _rep in range(repeat):
                    de = {"edge1": True, "gath1": "gath",
                          "ag": False, "fc": False}[variant]
                    one_layer(hT_t, rhs1_t, er1_sb, t1, f"1r{_rep}",
                              do_edge=de, do_ag=(variant != "fc"))
                repeat = 0  # skip the full pipeline below

            for _rep in range(repeat):
                with (
                    tc.tile_pool(name=f"postps{_rep}", bufs=2,
                                 space="PSUM") as postps,
                    tc.tile_pool(name=f"poststg{_rep}", bufs=3) as poststg,
                ):
                    slice2 = dpool.tile([RSLICE, TCOLS], dt.float32,
                                        tag=f"slice2r{_rep}")
                    tab2 = dpool.tile([TROWS, TCOLS], dt.float32,
                                      addr_space="Shared", tag=f"tab2r{_rep}")

                    xT_full = lhspool.tile([IN, NCOLS], dt.bfloat16,
                                           tag="lhs")
                    xT = xT_full[0:HID, :]

                    def post1(nb):
                        # stream layer-2 prep for finished block nb:
                        # transpose t1 -> lin1+relu -> fc2 -> slice2 row DMA
                        nc.sync.dma_start_transpose(
                            t1T[:, nb * P:(nb + 1) * P], t1[:, nb, :])
                        psA = postps.tile([HID, P], dt.float32, tag="post")
                        nc.tensor.matmul(psA[:], l1w_t[:],
                                         t1T[:, nb * P:(nb + 1) * P],
                                         start=True, stop=True)
                        nc.scalar.activation(xT[:, nb * P:(nb + 1) * P],
                                             psA[:],
                                             mybir.ActivationFunctionType.Relu,
                                             bias=b1_t[:])
                        psB = postps.tile([P, 132], dt.float32, tag="post")
                        nc.tensor.matmul(psB[:], xT[:, nb * P:(nb + 1) * P],
                                         rhs2_t[:], start=True, stop=True)
                        stg = poststg.tile([P, TCOLS], dt.float32, tag="pstg")
                        nc.scalar.activation(
                            stg[:].bitcast(dt.bfloat16)[:, 0:128],
                            psB[:, 0:128],
                            mybir.ActivationFunctionType.Copy)
                        nc.vector.tensor_copy(stg[:, 64:68], psB[:, 128:132])
                        nc.vector.tensor_copy(er2_sb[:, nb, :],
                                              psB[:, 130:132])
                        nc.sync.dma_start(
                            slice2[1 + nb * P:1 + (nb + 1) * P, :], stg[:])

                    def post2(nb):
                        # stream final output for finished block nb
                        nc.sync.dma_start_transpose(
                            t2T[:, nb * P:(nb + 1) * P], t2[:, nb, :])
                        ps = postps.tile([P, OUT], dt.float32, tag="post")
                        nc.tensor.matmul(ps[:], t2T[:, nb * P:(nb + 1) * P],
                                         l2w_t[:], start=True, stop=True)
                        og = poststg.tile([P, OUT], dt.float32, tag="og")
                        nc.vector.tensor_tensor(og[:], ps[:], b2_t[:],
                                                mybir.AluOpType.add)
                        r0 = nb * P
                        r1 = min(r0 + P, NSHARD)
                        if r1 > r0:
                            nc.sync.dma_start(d_out[r0:r1, :],
                                              og[0:r1 - r0, :])

                    one_layer(hT_t, rhs1_t, er1_sb, t1, f"1r{_rep}")
                    for nb in range(NBLKS):
                        post1(nb)
                    nc.sync.dma_start(slice2[0:1, :], pad_t[:])
                    nc.gpsimd.collective_compute(
                        "AllGather", mybir.AluOpType.bypass,
                        replica_groups=[list(range(NCORES))],
                        ins=[slice2.opt()], outs=[tab2.opt()])
                    with (
                        tc.tile_pool(name=f"gpool2{_rep}", bufs=4) as gpool2,
                        tc.tile_pool(name=f"epool2{_rep}", bufs=5) as epool2,
                        tc.tile_pool(name=f"pspool2{_rep}", bufs=3,
                                     space="PSUM") as psp2,
                        tc.tile_pool(name=f"epspool2{_rep}", bufs=3,
                                     space="PSUM") as epsp2,
                    ):
                        edge_phase(tab2, t2, er2_sb, gpool2, epool2, psp2,
                                   epsp2)
                    for nb in range(NBLKS):
                        post2(nb)

    nc.compile()
    return nc


def kernel(**inputs) -> np.ndarray:
    from concourse.bass_utils import run_bass_kernel_spmd

    args = {k: np.asarray(v) for k, v in inputs.items()}
    shared, per_core, sbh = preprocess(args)
    nc = build(sbh)
    in_maps = [{**shared, **pc} for pc in per_core]
    res = run_bass_kernel_spmd(nc, in_maps, list(range(NCORES)))
    out = np.concatenate([res.results[c]["out"] for c in range(NCORES)], axis=0)
    return np.ascontiguousarray(out.astype(np.float32))



# revision 46
# speedup vs baseline: 110.7042x; 110.7042x over previous
"""GAT 2-layer kernel for 8 Trainium2 NeuronCores (SPMD via bass/Tile).

Strategy:
  - dst-shard nodes across 8 cores; edges grouped by owner core, then by
    128-wide local dst block, then by src-table half, sorted by src.
  - Per layer a DRAM "table" holds one 512B row per node:
      [128 x bf16 feat | el0 el1 er0 er1 f32 | pad] = 128 f32 cols.
    fc is data-parallel over node shards; the table is replicated with TWO
    chunk-major AllGathers per layer (chunk == gather half) so half-0
    edge gathers only wait on the first chunk.
  - Edge phase: per dst block, TWO batched dma_gather ops (one per table
    half, int16 local indices) fetch all edge rows at once, spread
    round-robin over 4 SWDGE queues (queue parallelism hides the per-
    descriptor HBM latency; this is the single biggest win). Compute
    ex = exp(leakyrelu(el[src]+er[dst])), scale gathered feats, aggregate
    with one-hot matmuls (Pm^T @ [G*ex | ex]) accumulating in PSUM;
    normalize by the z columns afterwards.
  - Pm is built on DVE (pair-packed is_equal vs an iota constant); PmT for
    the er[dst] dispersal comes from PE transposes of Pm (batched 4 tiles
    per PSUM bank, ACT copies back) - no broadcast DMA.
  - post passes (lin1+relu+fc2 / lin2+out) transpose t via PE, not DMA.
  - Pad gather slots point at spread-out written rows and carry a dloc
    sentinel (255) whose one-hot column is all-zero, so they contribute
    exactly nothing.
  - Attention logits el/er are folded into the fc matmul via W@a products.
"""

import sys

import numpy as np

sys.path.insert(0, "/opt/trn_rl_repo")

import ml_dtypes

# problem sizes (overridable via configure() for scaled-down testing)
N = 50000
E = 800000
IN, HID, OUT = 128, 64, 64
H = 2
NEG = 0.2
NCORES = 8
P = 128
NSHARD = N // NCORES
NBLK = (NSHARD + P - 1) // P
RSLICE = NBLK * P + P         # table rows per rank slice (1 pad + nodes + junk)
TROWS = RSLICE * NCORES
HROWS = TROWS // 2            # rows per gather half (must fit int16)
TCOLS = 128                   # table row: 512B
PAD_EL = -100.0


def configure(n, e):
    """Adjust module globals for a scaled-down test configuration."""
    global N, E, NSHARD, NBLK, RSLICE, TROWS, HROWS
    N, E = n, e
    NSHARD = N // NCORES
    NBLK = (NSHARD + P - 1) // P
    RSLICE = NBLK * P + P
    TROWS = RSLICE * NCORES
    HROWS = TROWS // 2


CSPLIT = RSLICE // 2          # slice rows per AG chunk (chunk == gather half)


def _half_lrow(node):
    """chunk-major table: half h holds rows [core*CSPLIT + (loc - h*CSPLIT)]
    where loc = 1 + node % NSHARD is the row inside the core's slice."""
    c = node // NSHARD
    loc = 1 + (node % NSHARD)
    hh = (loc >= CSPLIT).astype(np.int64)
    return hh, c * CSPLIT + loc - hh * CSPLIT


def preprocess(a):
    bf16 = ml_dtypes.bfloat16
    f32 = np.float32
    h, src, dst = a["h"], a["src"].astype(np.int64), a["dst"].astype(np.int64)
    assert HROWS <= 32768

    def rhs_pack(W, al, ar):
        k = W.shape[0]
        Wr = W.reshape(k, H, W.shape[1] // H)
        wal = np.einsum("ihj,hj->ih", Wr, al)
        war = np.einsum("ihj,hj->ih", Wr, ar)
        return np.concatenate([W, wal, war], axis=1).astype(bf16)

    rhsW1 = rhs_pack(a["W1"].astype(f32), a["al1"].astype(f32), a["ar1"].astype(f32))
    rhsW2 = rhs_pack(a["W2"].astype(f32), a["al2"].astype(f32), a["ar2"].astype(f32))
    lin1_bp = a["lin1_b"].astype(f32) + a["b1"].astype(f32) @ a["lin1_W"].astype(f32)
    lin2_bp = a["lin2_b"].astype(f32) + a["b2"].astype(f32) @ a["lin2_W"].astype(f32)

    core = dst // NSHARD
    local = dst - core * NSHARD
    blk = local // P
    dloc = local % P
    half, lrow = _half_lrow(src)
    order = np.lexsort((src, half, blk, core))
    core_s, blk_s, half_s = core[order], blk[order], half[order]
    dloc_s, lrow_s = dloc[order], lrow[order]

    counts = np.zeros((NCORES, NBLK, 2), np.int64)
    np.add.at(counts, (core_s, blk_s, half_s), 1)
    sbh = -(-counts.max(axis=0) // P)          # [NBLK, 2] tiles per (blk, half)
    for b in range(NBLK):
        if sbh[b].sum() == 0:
            sbh[b, 0] = 1
    tiles_total = int(sbh.sum())
    slots_total = tiles_total * P
    tile_base = np.zeros((NBLK, 2), np.int64)  # global tile index of (b, h)
    acc = 0
    for b in range(NBLK):
        for hh in range(2):
            tile_base[b, hh] = acc
            acc += sbh[b, hh]

    # pad slots: spread across written rows of many cores/banks; dloc
    # sentinel 255 makes their Pm/PmT columns all-zero so they contribute
    # exactly nothing regardless of the (finite) data gathered.
    pad_rows = []           # per half: safe (written) rows, spread over cores
    for hh in range(2):
        if hh == 0:
            los = np.arange(1, CSPLIT)                  # real node rows
        else:
            los = np.arange(0, 1 + NBLK * P - CSPLIT)   # written rows
        rows = (np.arange(NCORES)[:, None] * CSPLIT + los[None, :]).T.ravel()
        pad_rows.append(rows.astype(np.int64))
    idx_arr = np.zeros((NCORES, slots_total), np.int16)
    for hh in range(2):
        pr = pad_rows[hh]
        for b in range(NBLK):
            s0 = int(tile_base[b, hh]) * P
            n_ = int(sbh[b, hh]) * P
            idx_arr[:, s0:s0 + n_] = pr[np.arange(n_) % len(pr)][None, :]
    dloc_arr = np.full((NCORES, slots_total), 255.0, np.float32)
    for c in range(NCORES):
        m = core_s == c
        cb, ch = blk_s[m], half_s[m]
        cd, cl = dloc_s[m], lrow_s[m]
        cnt = counts[c]                        # [NBLK, 2]
        off = 0
        for b in range(NBLK):
            for hh in range(2):
                n_ = int(cnt[b, hh])
                if n_:
                    s0 = int(tile_base[b, hh]) * P
                    idx_arr[c, s0:s0 + n_] = cl[off:off + n_]
                    dloc_arr[c, s0:s0 + n_] = cd[off:off + n_]
                    off += n_
    # wrapped int16 index layout: index j -> [j%16 (+16r), j//16]
    w16 = idx_arr.reshape(NCORES, slots_total // 16, 16).transpose(0, 2, 1)
    idx16 = np.tile(w16, (1, 8, 1)).astype(np.int16)
    dloc_pt = np.ascontiguousarray(
        dloc_arr.reshape(NCORES, tiles_total, P).transpose(0, 2, 1)).astype(bf16)
    dloc2_pt = np.ascontiguousarray(np.repeat(dloc_pt, 2, axis=2))

    NCOLS = NBLK * P
    hT = np.zeros((NCORES, IN, NCOLS), bf16)
    for c in range(NCORES):
        hT[c, :, :NSHARD] = h[c * NSHARD:(c + 1) * NSHARD].T.astype(bf16)

    padrow = np.zeros((1, TCOLS), f32)
    padrow[0, 64] = PAD_EL
    padrow[0, 65] = PAD_EL
    iota2d = np.tile(np.arange(P, dtype=f32), (P, 1)).astype(bf16)

    shared = dict(rhsW1=rhsW1, rhsW2=rhsW2,
                  lin1W=a["lin1_W"].astype(bf16), lin2W=a["lin2_W"].astype(bf16),
                  b1col=lin1_bp.reshape(HID, 1).astype(f32),
                  b2row=np.tile(lin2_bp[None, :], (P, 1)).astype(f32),
                  padrow=padrow, iota2d=iota2d,
                  ident=np.eye(P, dtype=bf16),
                  iotaP=np.arange(P, dtype=f32).reshape(P, 1))
    per_core = [dict(hT=np.ascontiguousarray(hT[c]), idx16=idx16[c],
                     dloc2=dloc2_pt[c])
                for c in range(NCORES)]
    return shared, per_core, sbh


def build(sbh, repeat=1, variant="all"):
    import concourse.bass as bass
    import concourse.bacc as bacc
    import concourse.mybir as mybir
    from concourse import tile

    dt = mybir.dt
    NBLKS = NBLK
    sbh = np.asarray(sbh)
    tiles_total = int(sbh.sum())
    slots_total = tiles_total * P
    tile_base = np.zeros((NBLK, 2), np.int64)
    acc = 0
    for b in range(NBLK):
        for hh in range(2):
            tile_base[b, hh] = acc
            acc += sbh[b, hh]
    NCOLS = NBLK * P

    nc = bacc.Bacc("TRN2", target_bir_lowering=False, debug=False,
                   num_devices=NCORES, num_swdge_queues=4)

    def din(name, shape, dty):
        return nc.dram_tensor(name, shape, dty, kind="ExternalInput").ap()

    d_hT = din("hT", [IN, NCOLS], dt.bfloat16)
    d_idx16 = din("idx16", [P, slots_total // 16], dt.int16)
    d_dloc2 = din("dloc2", [P, 2 * tiles_total], dt.bfloat16)
    d_rhsW1 = din("rhsW1", [IN, 132], dt.bfloat16)
    d_rhsW2 = din("rhsW2", [HID, 132], dt.bfloat16)
    d_lin1W = din("lin1W", [2 * HID, HID], dt.bfloat16)
    d_lin2W = din("lin2W", [2 * OUT, OUT], dt.bfloat16)
    d_b1col = din("b1col", [HID, 1], dt.float32)
    d_b2row = din("b2row", [P, OUT], dt.float32)
    d_padrow = din("padrow", [1, TCOLS], dt.float32)
    d_iota = din("iota2d", [P, P], dt.bfloat16)
    d_ident = din("ident", [P, P], dt.bfloat16)
    d_iotaP = din("iotaP", [P, 1], dt.float32)
    d_out = nc.dram_tensor("out", [NSHARD, OUT], dt.float32,
                           kind="ExternalOutput").ap()

    with tile.TileContext(nc) as tc:
        with (
            tc.tile_pool(name="const", bufs=1) as cpool,
            tc.tile_pool(name="dram", bufs=1, space="DRAM") as dpool,
            tc.tile_pool(name="big", bufs=1) as bigpool,
            tc.tile_pool(name="lhs", bufs=2 if repeat > 1 else 1) as lhspool,
        ):
            idx16_t = cpool.tile([P, slots_total // 16], dt.int16)
            dloc2_t = cpool.tile([P, 2 * tiles_total], dt.bfloat16)
            iota_t = cpool.tile([P, P], dt.bfloat16)
            ident_t = cpool.tile([P, P], dt.bfloat16)
            iotaP_t = cpool.tile([P, 1], dt.float32)
            b2_t = cpool.tile([P, OUT], dt.float32)
            b1_t = cpool.tile([HID, 1], dt.float32)
            pad_t = cpool.tile([1, TCOLS], dt.float32)
            rhs1_t = cpool.tile([IN, 132], dt.bfloat16)
            rhs2_t = cpool.tile([HID, 132], dt.bfloat16)
            l1w_t = cpool.tile([2 * HID, HID], dt.bfloat16)
            l2w_t = cpool.tile([2 * OUT, OUT], dt.bfloat16)
            for t, d in ((idx16_t, d_idx16), (dloc2_t, d_dloc2),
                         (iota_t, d_iota), (ident_t, d_ident),
                         (iotaP_t, d_iotaP),
                         (b2_t, d_b2row), (b1_t, d_b1col), (pad_t, d_padrow),
                         (rhs1_t, d_rhsW1), (rhs2_t, d_rhsW2),
                         (l1w_t, d_lin1W), (l2w_t, d_lin2W)):
                nc.sync.dma_start(t[:], d[:])

            hT_t = lhspool.tile([IN, NCOLS], dt.bfloat16, tag="lhs")
            nc.sync.dma_start(hT_t[:], d_hT[:])

            er1_sb = bigpool.tile([P, NBLKS, H], dt.bfloat16)
            er2_sb = bigpool.tile([P, NBLKS, H], dt.bfloat16)
            t1 = bigpool.tile([P, NBLKS, P], dt.bfloat16)
            t1T = bigpool.tile([P, NCOLS], dt.bfloat16)
            t2 = bigpool.tile([P, NBLKS, P], dt.bfloat16)
            t2T = t1T

            def fc_phase(lhs_tile, rhs_t, slice_d, er_sb, pool_fc, pool_stg):
                nc.sync.dma_start(slice_d[0:1, :], pad_t[:])
                for nb in range(NBLKS):
                    ps = pool_fc.tile([P, 132], dt.float32, tag="fcps")
                    nc.tensor.matmul(ps[:], lhs_tile[:, nb * P:(nb + 1) * P],
                                     rhs_t[:], start=True, stop=True)
                    stg = pool_stg.tile([P, TCOLS], dt.float32, tag="fcstg")
                    nc.scalar.activation(
                        stg[:].bitcast(dt.bfloat16)[:, 0:128], ps[:, 0:128],
                        mybir.ActivationFunctionType.Copy)
                    nc.vector.tensor_copy(stg[:, 64:68], ps[:, 128:132])
                    nc.vector.tensor_copy(er_sb[:, nb, :], ps[:, 130:132])
                    nc.sync.dma_start(
                        slice_d[1 + nb * P:1 + (nb + 1) * P, :], stg[:])

            def edge_phase(tab_d, t_stg, er_sb, pool_g, pool_e, pool_ps,
                           pool_eps, pool_tp, post_block=None, skip_gather=False):
                qi = 0
                for b in range(NBLKS):
                    S0, S1 = int(sbh[b, 0]), int(sbh[b, 1])
                    S = S0 + S1
                    t0 = int(tile_base[b, 0])
                    G = pool_g.tile([P, S, TCOLS], dt.float32, tag="g")
                    for hh, (sh, toff) in enumerate(((S0, 0), (S1, S0))):
                        if sh == 0 or skip_gather:
                            continue
                        s0 = (t0 + toff) * P
                        nc.gpsimd.dma_gather(
                            G[:, toff:toff + sh, :],
                            tab_d[hh][:, :],
                            idx16_t[:, s0 // 16:(s0 + sh * P) // 16],
                            sh * P, sh * P, TCOLS, single_packet=False,
                            queue_num=qi % 4)
                        qi += 1
                    Pm = pool_e.tile([P, S, P], dt.bfloat16, tag="pm")
                    nc.vector.tensor_tensor(
                        Pm[:].rearrange("p s (d two) -> p s d two", two=2),
                        dloc2_t[:, 2 * t0:2 * (t0 + S)]
                        .rearrange("p (s two) -> p s two", two=2)
                        .unsqueeze(2).broadcast_to([P, S, 64, 2]),
                        iota_t[:].rearrange("p (d two) -> p d two", two=2)
                        .unsqueeze(1).broadcast_to([P, S, 64, 2]),
                        mybir.AluOpType.is_equal)
                    PmT = pool_e.tile([P, S, P], dt.bfloat16, tag="pmt")
                    for s4 in range(0, S, 4):
                        sn = min(4, S - s4)
                        psT = pool_tp.tile([P, 4 * P], dt.bfloat16, tag="ptr")
                        for k in range(sn):
                            nc.tensor.transpose(psT[:, k * P:(k + 1) * P],
                                                Pm[:, s4 + k, :], ident_t[:])
                        nc.scalar.activation(
                            PmT[:, s4:s4 + sn, :]
                            .rearrange("p s e -> p (s e)"),
                            psT[:, 0:sn * P],
                            mybir.ActivationFunctionType.Copy)
                    er_ps = pool_eps.tile([P, H * S], dt.float32, tag="erps")
                    for s in range(S):
                        nc.tensor.matmul(er_ps[:, H * s:H * (s + 1)],
                                         PmT[:, s, :], er_sb[:, b, :],
                                         start=True, stop=True)
                    ex = pool_e.tile([P, S, H], dt.float32, tag="ex")
                    tmp = pool_e.tile([P, S, H], dt.float32, tag="tmp")
                    nc.vector.tensor_tensor(
                        ex[:], G[:, :, 64:66],
                        er_ps[:].rearrange("p (s h) -> p s h", s=S),
                        mybir.AluOpType.add)
                    nc.vector.tensor_scalar(tmp[:], ex[:], NEG, None,
                                            mybir.AluOpType.mult)
                    nc.vector.tensor_tensor(ex[:], ex[:], tmp[:],
                                            mybir.AluOpType.max)
                    stg = pool_e.tile([P, S, 132], dt.bfloat16, tag="stg")
                    exd = stg[:, :, 128:132].rearrange(
                        "p s (h two) -> p s h two", h=H)
                    for k in range(2):
                        nc.scalar.activation(exd[:, :, :, k], ex[:],
                                             mybir.ActivationFunctionType.Exp)
                    Gb = G[:].bitcast(dt.bfloat16)
                    for hh in range(H):
                        nc.vector.tensor_tensor(
                            stg[:, :, hh * 64:(hh + 1) * 64]
                            .rearrange("p s (c two) -> p s c two", two=2),
                            Gb[:, :, hh * 64:(hh + 1) * 64]
                            .rearrange("p s (c two) -> p s c two", two=2),
                            stg[:, :, 128 + 2 * hh:130 + 2 * hh]
                            .rearrange("p s (c two) -> p s c two", two=2)
                            .broadcast_to([P, S, 32, 2]),
                            mybir.AluOpType.mult)
                    acc = pool_ps.tile([P, 132], dt.float32, tag="acc")
                    for s in range(S):
                        nc.tensor.matmul(acc[:], Pm[:, s, :], stg[:, s, :],
                                         start=(s == 0), stop=(s == S - 1))
                    rz = pool_e.tile([P, H], dt.float32, tag="rz")
                    nc.vector.tensor_scalar(
                        rz[:],
                        acc[:, 128:132].rearrange("p (h two) -> p h two", h=H)
                        [:, :, 0:1].squeeze(2),
                        1e-30, None, mybir.AluOpType.add)
                    nc.vector.reciprocal(rz[:], rz[:])
                    for hh in range(H):
                        nc.vector.tensor_scalar(
                            t_stg[:, b, hh * 64:(hh + 1) * 64],
                            acc[:, hh * 64:(hh + 1) * 64],
                            rz[:, hh:hh + 1], None, mybir.AluOpType.mult)
                    if post_block is not None:
                        post_block(b)

            def gather_only(tab_d, pool_g, rep, nq=1, sp=False, ecols=TCOLS):
                qi = 0
                for b in range(NBLKS):
                    S0, S1 = int(sbh[b, 0]), int(sbh[b, 1])
                    S = S0 + S1
                    t0 = int(tile_base[b, 0])
                    G = pool_g.tile([P, S, ecols], dt.float32, tag="g")
                    for hh, (sh, toff) in enumerate(((S0, 0), (S1, S0))):
                        if sh == 0:
                            continue
                        s0 = (t0 + toff) * P
                        nc.gpsimd.dma_gather(
                            G[:, toff:toff + sh, :],
                            tab_d[hh][:, 0:ecols],
                            idx16_t[:, s0 // 16:(s0 + sh * P) // 16],
                            sh * P, sh * P, ecols,
                            elem_step=TCOLS if ecols != TCOLS else None,
                            single_packet=sp, queue_num=qi % nq)
                        qi += 1
                    jk = pool_g.tile([P, 1], dt.float32, tag="jk")
                    nc.vector.tensor_copy(jk[:], G[:, 0, 0:1])

            def one_layer(lhs_tile, rhs_t, er_sb, t_stg, tag, do_edge=True,
                          do_ag=True, post_block=None, pool_tp=None):
                slice_d = dpool.tile([RSLICE, TCOLS], dt.float32,
                                     tag=f"slice{tag}")
                tabA = dpool.tile([HROWS, TCOLS], dt.float32,
                                  addr_space="Shared", tag=f"tabA{tag}")
                tabB = dpool.tile([HROWS, TCOLS], dt.float32,
                                  addr_space="Shared", tag=f"tabB{tag}")
                tab_d = (tabA, tabB)
                with (
                    tc.tile_pool(name=f"fcps{tag}", bufs=2,
                                 space="PSUM") as fcps,
                    tc.tile_pool(name=f"fcstg{tag}", bufs=3) as fcstg,
                ):
                    fc_phase(lhs_tile, rhs_t, slice_d, er_sb, fcps, fcstg)
                if not do_ag:
                    return None
                CS = RSLICE // 2
                nc.gpsimd.collective_compute(
                    "AllGather", mybir.AluOpType.bypass,
                    replica_groups=[list(range(NCORES))],
                    ins=[slice_d[0:CS, :].opt()], outs=[tabA.opt()])
                nc.gpsimd.collective_compute(
                    "AllGather", mybir.AluOpType.bypass,
                    replica_groups=[list(range(NCORES))],
                    ins=[slice_d[CS:RSLICE, :].opt()], outs=[tabB.opt()])
                if not do_edge:
                    return None
                from contextlib import ExitStack
                with (
                    tc.tile_pool(name=f"gpool{tag}", bufs=5) as gpool,
                    tc.tile_pool(name=f"epool{tag}", bufs=5) as epool,
                    tc.tile_pool(name=f"pspool{tag}", bufs=2,
                                 space="PSUM") as psp,
                    tc.tile_pool(name=f"epspool{tag}", bufs=2,
                                 space="PSUM") as epsp,
                    ExitStack() as _tpctx,
                ):
                    tp = pool_tp
                    if tp is None:
                        tp = _tpctx.enter_context(
                            tc.tile_pool(name=f"tppool{tag}", bufs=2,
                                         space="PSUM"))
                    if isinstance(do_edge, tuple):
                        gather_only(tab_d, gpool, tag, *do_edge)
                    elif do_edge == "gath":
                        gather_only(tab_d, gpool, tag)
                    else:
                        edge_phase(tab_d, t_stg, er_sb, gpool, epool, psp,
                                   epsp, tp, post_block=post_block,
                                   skip_gather=(do_edge == "ng"))
                return tab_d

            if variant in ("edge1", "gath1", "ng1", "ag", "fc") or variant.startswith("gx"):
                for _rep in range(repeat):
                    if variant.startswith("gx"):
                        # gx<nq>_<sp>_<ecols>  e.g. gx4_0_128, gx1_1_64
                        nq, sp, ecols = variant[2:].split("_")
                        de = (int(nq), bool(int(sp)), int(ecols))
                    else:
                        de = {"edge1": True, "gath1": "gath", "ng1": "ng",
                              "ag": False, "fc": False}[variant]
                    one_layer(hT_t, rhs1_t, er1_sb, t1, f"1r{_rep}",
                              do_edge=de, do_ag=(variant != "fc"))
                repeat = 0  # skip the full pipeline below

            stage = {"l1": 0, "l1p": 1, "l1pa": 2, "l2": 3, "all": 4}.get(
                variant, 4)
            for _rep in range(repeat):
                with (
                    tc.tile_pool(name=f"postps{_rep}", bufs=2,
                                 space="PSUM") as postps,
                    tc.tile_pool(name=f"tpps{_rep}", bufs=2,
                                 space="PSUM") as tpps,
                    tc.tile_pool(name=f"poststg{_rep}", bufs=3) as poststg,
                ):
                    slice2 = dpool.tile([RSLICE, TCOLS], dt.float32,
                                        tag=f"slice2r{_rep}")
                    tab2A = dpool.tile([HROWS, TCOLS], dt.float32,
                                       addr_space="Shared", tag=f"tab2Ar{_rep}")
                    tab2B = dpool.tile([HROWS, TCOLS], dt.float32,
                                       addr_space="Shared", tag=f"tab2Br{_rep}")
                    tab2 = (tab2A, tab2B)

                    xT_full = lhspool.tile([IN, NCOLS], dt.bfloat16,
                                           tag="lhs")
                    xT = xT_full[0:HID, :]

                    def post1(nb):
                        # stream layer-2 prep for finished block nb:
                        # transpose t1 -> lin1+relu -> fc2 -> slice2 row DMA
                        psT = tpps.tile([P, 4 * P], dt.bfloat16, tag="ptr")
                        nc.tensor.transpose(psT[:, 0:P], t1[:, nb, :],
                                            ident_t[:])
                        nc.scalar.activation(
                            t1T[:, nb * P:(nb + 1) * P], psT[:, 0:P],
                            mybir.ActivationFunctionType.Copy)
                        psA = postps.tile([HID, P], dt.float32, tag="post")
                        nc.tensor.matmul(psA[:], l1w_t[:],
                                         t1T[:, nb * P:(nb + 1) * P],
                                         start=True, stop=True)
                        nc.scalar.activation(xT[:, nb * P:(nb + 1) * P],
                                             psA[:],
                                             mybir.ActivationFunctionType.Relu,
                                             bias=b1_t[:])
                        psB = postps.tile([P, 132], dt.float32, tag="post")
                        nc.tensor.matmul(psB[:], xT[:, nb * P:(nb + 1) * P],
                                         rhs2_t[:], start=True, stop=True)
                        stg = poststg.tile([P, TCOLS], dt.float32, tag="pstg")
                        nc.scalar.activation(
                            stg[:].bitcast(dt.bfloat16)[:, 0:128],
                            psB[:, 0:128],
                            mybir.ActivationFunctionType.Copy)
                        nc.vector.tensor_copy(stg[:, 64:68], psB[:, 128:132])
                        nc.vector.tensor_copy(er2_sb[:, nb, :],
                                              psB[:, 130:132])
                        nc.sync.dma_start(
                            slice2[1 + nb * P:1 + (nb + 1) * P, :], stg[:])

                    def post2(nb):
                        # stream final output for finished block nb
                        psT = tpps.tile([P, 4 * P], dt.bfloat16, tag="ptr")
                        nc.tensor.transpose(psT[:, 0:P], t2[:, nb, :],
                                            ident_t[:])
                        nc.scalar.activation(
                            t2T[:, nb * P:(nb + 1) * P], psT[:, 0:P],
                            mybir.ActivationFunctionType.Copy)
                        ps = postps.tile([P, OUT], dt.float32, tag="post")
                        nc.tensor.matmul(ps[:], t2T[:, nb * P:(nb + 1) * P],
                                         l2w_t[:], start=True, stop=True)
                        og = poststg.tile([P, OUT], dt.float32, tag="og")
                        nc.vector.tensor_tensor(og[:], ps[:], b2_t[:],
                                                mybir.AluOpType.add)
                        r0 = nb * P
                        r1 = min(r0 + P, NSHARD)
                        if r1 > r0:
                            nc.sync.dma_start(d_out[r0:r1, :],
                                              og[0:r1 - r0, :])

                    one_layer(hT_t, rhs1_t, er1_sb, t1, f"1r{_rep}",
                              pool_tp=tpps)
                    if stage >= 1:
                        for nb in range(NBLKS):
                            post1(nb)
                    if stage < 2:
                        continue
                    nc.sync.dma_start(slice2[0:1, :], pad_t[:])
                    CS2 = RSLICE // 2
                    nc.gpsimd.collective_compute(
                        "AllGather", mybir.AluOpType.bypass,
                        replica_groups=[list(range(NCORES))],
                        ins=[slice2[0:CS2, :].opt()], outs=[tab2A.opt()])
                    nc.gpsimd.collective_compute(
                        "AllGather", mybir.AluOpType.bypass,
                        replica_groups=[list(range(NCORES))],
                        ins=[slice2[CS2:RSLICE, :].opt()], outs=[tab2B.opt()])
                    if stage < 3:
                        continue
                    with (
                        tc.tile_pool(name=f"gpool2{_rep}",
                                     bufs=5) as gpool2,
                        tc.tile_pool(name=f"epool2{_rep}", bufs=5) as epool2,
                        tc.tile_pool(name=f"pspool2{_rep}", bufs=2,
                                     space="PSUM") as psp2,
                        tc.tile_pool(name=f"epspool2{_rep}", bufs=2,
                                     space="PSUM") as epsp2,
                    ):
                        edge_phase(tab2, t2, er2_sb, gpool2, epool2, psp2,
                                   epsp2, tpps)
                    if stage >= 4:
                        for nb in range(NBLKS):
                            post2(nb)

    nc.compile()
    return nc


def kernel(**inputs) -> np.ndarray:
    from concourse.bass_utils import run_bass_kernel_spmd

    args = {k: np.asarray(v) for k, v in inputs.items()}
    shared, per_core, sbh = preprocess(args)
    nc = build(sbh)
    in_maps = [{**shared, **pc} for pc in per_core]
    res = run_bass_kernel_spmd(nc, in_maps, list(range(NCORES)))
    out = np.concatenate([res.results[c]["out"] for c in range(NCORES)], axis=0)
    return np.ascontiguousarray(out.astype(np.float32))

